# revision 7
# baseline (speedup 1.0000x reference)
"""GroupedExpertNetwork (SwiGLU per-expert MLP) Trainium2 kernel.

Expert-parallel: 8 experts -> 8 NeuronCores, one expert per core.
Per core:  g = x @ gate; u = x @ inner; h = silu(g)*u; out = h @ outp
Shapes per expert: x [T=2048, D=2048], gate/inner [D, I=4096], outp [I, D].

Low-HBM-traffic structure (109 MB/core vs 227 MB for the naive T-blocked
version), while keeping the PE stream dense at the bf16 roofline:

- Phase 1 (per T-half of 1024): for each i-tile (128 cols of gate/inner),
  run weight-stationary accumulation chains over both 512-token blocks of
  the half. Weights are streamed twice total (once per half) instead of
  once per 512-token block. xT stays resident as 16 per-k tiles per half.
  h^T [I, T] is materialized fully in SBUF (128 KB/partition-slice, bf16).
- Phase 2 (full T): output-projection-stationary: lhsT = ow [i, d-tile]
  chunks, moving = h^T columns. ow is streamed exactly once. This yields
  out^T [D, T] on the device; the host transposes back (cheap).

All matmul free dims are 512 (one PSUM bank), bf16 inputs, fp32 PSUM
accumulate. One shared 8-bank PSUM pool cycles through phase-1 g/u chains
and phase-2 out chains.
"""

import numpy as np
import ml_dtypes

E, T, D, I = 8, 2048, 2048, 4096
P = 128
TH = 1024                # T half for phase 1
NH = T // TH             # 2 halves
NTB = TH // 512          # 2 512-blocks per half
KD = D // P              # 16 contraction chunks (first layer)
KI = I // P              # 32 contraction chunks (second layer)
NMI = I // P             # 32 i-tiles (phase 1 output tiles)
NDT = D // P             # 16 d-tiles (phase 2 output tiles)

_COMPILED = None


def _build_program(skip_mm=False, skip_dma=False):
    import concourse.mybir as mybir
    import concourse.tile as tile
    from concourse import bacc

    bf16 = mybir.dt.bfloat16
    f32 = mybir.dt.float32

    nc = bacc.Bacc(
        "TRN2",
        target_bir_lowering=False,
        debug=False,
        num_devices=E,
    )

    # Packed DRAM inputs (per core = one expert):
    # xt:  [NH, KD, P, TH]    x^T tiles; d = k*128+p, t = h*1024 + tt
    # gw:  [NMI, P, KD, P]    gate tiles;  [mi, p(d), k, i-col]
    # uw:  [NMI, P, KD, P]    inner tiles
    # ow:  [NDT, P, KI, P]    output-proj tiles; [dt, p(i), k, d-col]
    xt_d = nc.dram_tensor("xt", (NH, KD, P, TH), bf16, kind="ExternalInput")
    gw_d = nc.dram_tensor("gw", (NMI, P, KD, P), bf16, kind="ExternalInput")
    uw_d = nc.dram_tensor("uw", (NMI, P, KD, P), bf16, kind="ExternalInput")
    ow_d = nc.dram_tensor("ow", (NDT, P, KI, P), bf16, kind="ExternalInput")
    # out^T [D, T]; host transposes back to [T, D]
    out_d = nc.dram_tensor("out", (D, T), f32, kind="ExternalOutput")

    xt_ap = xt_d.ap()
    gw_ap = gw_d.ap()
    uw_ap = uw_d.ap()
    ow_ap = ow_d.ap()
    out_ap = out_d.ap().rearrange("(dt p) t -> dt p t", p=P)

    with tile.TileContext(nc) as tc:
        with (
            tc.tile_pool(name="xk", bufs=18) as xk_pool,
            tc.tile_pool(name="w", bufs=2) as w_pool,
            tc.tile_pool(name="ow", bufs=2) as ow_pool,
            tc.tile_pool(name="ht", bufs=1) as ht_pool,
            tc.tile_pool(name="tmp", bufs=2) as tmp_pool,
            tc.tile_pool(name="osb", bufs=2) as osb_pool,
            tc.tile_pool(name="ps", bufs=8, space="PSUM") as ps_pool,
        ):
            # h^T, fully resident: [p(i), ki, t]
            ht = ht_pool.tile([P, KI, T], bf16, tag="ht")

            def phase2_chains(dt, tbs):
                ow = ow_pool.tile([P, KI, P], bf16, tag="ow")
                if not skip_dma:
                    nc.sync.dma_start(ow[:], ow_ap[dt])
                for tb in tbs:
                    po = ps_pool.tile([P, 512], f32, tag="ps")
                    if not skip_mm:
                        for k in range(KI):
                            nc.tensor.matmul(
                                po[:],
                                ow[:, k, :],
                                ht[:, k, tb * 512:(tb + 1) * 512],
                                start=(k == 0),
                                stop=(k == KI - 1),
                            )
                        osb = osb_pool.tile([P, 512], f32, tag="osb")
                        nc.vector.tensor_copy(osb[:], po[:])
                        nc.sync.dma_start(
                            out_ap[dt, :, tb * 512:(tb + 1) * 512],
                            osb[:],
                        )

            # ---------------- Phase 1: g/u matmuls + silu*mul ----------------
            for h in range(NH):
                # Critical path first: mi=0 weights + first x tile, so the
                # first matmul chain can start as soon as possible; the
                # remaining x tiles stream behind it. The first gw tile is
                # split into k-quarters so chain 0 starts after ~1KB/part.
                gw0 = w_pool.tile([P, KD, P], bf16, tag="gw")
                uw0 = w_pool.tile([P, KD, P], bf16, tag="uw")
                xk = []
                if h == 0 and not skip_dma:
                    xt = xk_pool.tile([P, TH], bf16, tag="xk")
                    nc.sync.dma_start(xt[:], xt_ap[h, 0])
                    xk.append(xt)
                    KQ = KD // 4
                    for q in range(4):
                        nc.sync.dma_start(
                            gw0[:, q * KQ:(q + 1) * KQ, :],
                            gw_ap[0][:, q * KQ:(q + 1) * KQ, :],
                        )
                    nc.sync.dma_start(uw0[:], uw_ap[0])
                elif not skip_dma:
                    nc.sync.dma_start(gw0[:], gw_ap[0])
                    nc.sync.dma_start(uw0[:], uw_ap[0])
                for k in range(len(xk), KD):
                    xt = xk_pool.tile([P, TH], bf16, tag="xk")
                    if not skip_dma:
                        nc.sync.dma_start(xt[:], xt_ap[h, k])
                    xk.append(xt)

                for mi in range(NMI):
                    if mi == 0:
                        gw, uw = gw0, uw0
                    else:
                        gw = w_pool.tile([P, KD, P], bf16, tag="gw")
                        uw = w_pool.tile([P, KD, P], bf16, tag="uw")
                        if not skip_dma:
                            nc.sync.dma_start(gw[:], gw_ap[mi])
                            nc.sync.dma_start(uw[:], uw_ap[mi])

                    pg = []
                    pu = []
                    for tb in range(NTB):
                        ps = ps_pool.tile([P, 512], f32, tag="ps")
                        if not skip_mm:
                            for k in range(KD):
                                nc.tensor.matmul(
                                    ps[:],
                                    gw[:, k, :],
                                    xk[k][:, tb * 512:(tb + 1) * 512],
                                    start=(k == 0),
                                    stop=(k == KD - 1),
                                )
                        pg.append(ps)
                    for tb in range(NTB):
                        ps = ps_pool.tile([P, 512], f32, tag="ps")
                        if not skip_mm:
                            for k in range(KD):
                                nc.tensor.matmul(
                                    ps[:],
                                    uw[:, k, :],
                                    xk[k][:, tb * 512:(tb + 1) * 512],
                                    start=(k == 0),
                                    stop=(k == KD - 1),
                                )
                        pu.append(ps)
                    if not skip_mm:
                        for tb in range(NTB):
                            tmp = tmp_pool.tile([P, 512], f32, tag="tmp")
                            nc.scalar.activation(
                                tmp[:], pg[tb][:],
                                mybir.ActivationFunctionType.Silu,
                            )
                            t0 = h * TH + tb * 512
                            nc.vector.tensor_tensor(
                                ht[:, mi, t0:t0 + 512],
                                tmp[:],
                                pu[tb][:],
                                mybir.AluOpType.mult,
                            )

                if h == 0:
                    # Interleave dt=0's first-half out-chains here: they only
                    # need half-0's ht, and give the PE ~14us of work while
                    # half 1's x tiles reclaim SBUF slots from half 0 (their
                    # DMAs can't start earlier, so without this the PE would
                    # briefly stall at the half boundary).
                    phase2_chains(0, (0, 1))

            # ---------------- Phase 2: out^T = ow^T-stationary @ h^T ---------
            phase2_chains(0, (2, 3))
            for dt in range(1, NDT):
                phase2_chains(dt, range(T // 512))

    nc.compile()
    return nc


def _get_program():
    global _COMPILED
    if _COMPILED is None:
        _COMPILED = _build_program()
    return _COMPILED


def _pack_inputs(x, gate_proj, inner_proj, output_proj):
    bf16 = ml_dtypes.bfloat16
    in_maps = []
    for e in range(E):
        # x [T, D] -> xT tiles [NH, KD, P, TH]: xt[h,k,p,tt] = x[h*TH+tt, k*P+p]
        xt = np.ascontiguousarray(x[e].T).astype(bf16)      # [D, T]
        xt = xt.reshape(KD, P, NH, TH).transpose(2, 0, 1, 3)
        xt = np.ascontiguousarray(xt)
        # gate [D, I] -> [NMI, P, KD, P]: gw[mi,p,k,ii] = gate[k*P+p, mi*P+ii]
        gw = gate_proj[e].astype(bf16).reshape(KD, P, NMI, P).transpose(2, 1, 0, 3)
        gw = np.ascontiguousarray(gw)
        uw = inner_proj[e].astype(bf16).reshape(KD, P, NMI, P).transpose(2, 1, 0, 3)
        uw = np.ascontiguousarray(uw)
        # outp [I, D] -> [NDT, P, KI, P]: ow[dt,p,k,dd] = outp[k*P+p, dt*P+dd]
        ow = output_proj[e].astype(bf16).reshape(KI, P, NDT, P).transpose(2, 1, 0, 3)
        ow = np.ascontiguousarray(ow)
        in_maps.append({"xt": xt, "gw": gw, "uw": uw, "ow": ow})
    return in_maps


def kernel(x, gate_proj, inner_proj, output_proj, _trace=False, _trace_kwargs=None):
    from concourse import bass_utils

    nc = _get_program()
    in_maps = _pack_inputs(
        np.asarray(x), np.asarray(gate_proj), np.asarray(inner_proj),
        np.asarray(output_proj),
    )
    res = bass_utils.run_bass_kernel_spmd(
        nc,
        in_maps,
        core_ids=list(range(E)),
        trace=_trace,
        **(_trace_kwargs or {}),
    )
    # device output is out^T [D, T]; transpose back per expert
    out = np.stack(
        [np.ascontiguousarray(np.asarray(res.results[e]["out"]).T) for e in range(E)]
    )
    if _trace:
        return out.astype(np.float32, copy=False), res
    return out.astype(np.float32, copy=False)


# revision 8
# speedup vs baseline: 1.0010x; 1.0010x over previous
"""GroupedExpertNetwork (SwiGLU per-expert MLP) Trainium2 kernel.

Expert-parallel: 8 experts -> 8 NeuronCores, one expert per core.
Per core:  g = x @ gate; u = x @ inner; h = silu(g)*u; out = h @ outp
Shapes per expert: x [T=2048, D=2048], gate/inner [D, I=4096], outp [I, D].

Low-HBM-traffic structure (109 MB/core vs 227 MB for the naive T-blocked
version), while keeping the PE stream dense at the bf16 roofline:

- Phase 1 (per T-half of 1024): for each i-tile (128 cols of gate/inner),
  run weight-stationary accumulation chains over both 512-token blocks of
  the half. Weights are streamed twice total (once per half) instead of
  once per 512-token block. xT stays resident as 16 per-k tiles per half.
  h^T [I, T] is materialized fully in SBUF (128 KB/partition-slice, bf16).
- Phase 2 (full T): output-projection-stationary: lhsT = ow [i, d-tile]
  chunks, moving = h^T columns. ow is streamed exactly once. This yields
  out^T [D, T] on the device; the host transposes back (cheap).

All matmul free dims are 512 (one PSUM bank), bf16 inputs, fp32 PSUM
accumulate. One shared 8-bank PSUM pool cycles through phase-1 g/u chains
and phase-2 out chains.
"""

import numpy as np
import ml_dtypes

E, T, D, I = 8, 2048, 2048, 4096
P = 128
TH = 1024                # T half for phase 1
NH = T // TH             # 2 halves
NTB = TH // 512          # 2 512-blocks per half
KD = D // P              # 16 contraction chunks (first layer)
KI = I // P              # 32 contraction chunks (second layer)
NMI = I // P             # 32 i-tiles (phase 1 output tiles)
NDT = D // P             # 16 d-tiles (phase 2 output tiles)

_COMPILED = None


def _build_program(skip_mm=False, skip_dma=False):
    import concourse.mybir as mybir
    import concourse.tile as tile
    from concourse import bacc

    bf16 = mybir.dt.bfloat16
    f32 = mybir.dt.float32

    nc = bacc.Bacc(
        "TRN2",
        target_bir_lowering=False,
        debug=False,
        num_devices=E,
    )

    # Packed DRAM inputs (per core = one expert):
    # xt:  [NH, KD, P, TH]    x^T tiles; d = k*128+p, t = h*1024 + tt
    # gw:  [NMI, P, KD, P]    gate tiles;  [mi, p(d), k, i-col]
    # uw:  [NMI, P, KD, P]    inner tiles
    # ow:  [NDT, P, KI, P]    output-proj tiles; [dt, p(i), k, d-col]
    xt_d = nc.dram_tensor("xt", (NH, KD, P, TH), bf16, kind="ExternalInput")
    gw_d = nc.dram_tensor("gw", (NMI, P, KD, P), bf16, kind="ExternalInput")
    uw_d = nc.dram_tensor("uw", (NMI, P, KD, P), bf16, kind="ExternalInput")
    ow_d = nc.dram_tensor("ow", (NDT, P, KI, P), bf16, kind="ExternalInput")
    # out^T [D, T]; host transposes back to [T, D]
    out_d = nc.dram_tensor("out", (D, T), f32, kind="ExternalOutput")

    xt_ap = xt_d.ap()
    gw_ap = gw_d.ap()
    uw_ap = uw_d.ap()
    ow_ap = ow_d.ap()
    out_ap = out_d.ap().rearrange("(dt p) t -> dt p t", p=P)

    with tile.TileContext(nc) as tc:
        with (
            tc.tile_pool(name="xk", bufs=18) as xk_pool,
            tc.tile_pool(name="w", bufs=2) as w_pool,
            tc.tile_pool(name="ow", bufs=2) as ow_pool,
            tc.tile_pool(name="ht", bufs=1) as ht_pool,
            tc.tile_pool(name="tmp", bufs=2) as tmp_pool,
            tc.tile_pool(name="osb", bufs=2) as osb_pool,
            tc.tile_pool(name="ps", bufs=8, space="PSUM") as ps_pool,
        ):
            # h^T, fully resident: [p(i), ki, t]
            ht = ht_pool.tile([P, KI, T], bf16, tag="ht")

            def phase2_chains(dt, tbs):
                ow = ow_pool.tile([P, KI, P], bf16, tag="ow")
                if not skip_dma:
                    nc.sync.dma_start(ow[:], ow_ap[dt])
                for tb in tbs:
                    po = ps_pool.tile([P, 512], f32, tag="ps")
                    if not skip_mm:
                        for k in range(KI):
                            nc.tensor.matmul(
                                po[:],
                                ow[:, k, :],
                                ht[:, k, tb * 512:(tb + 1) * 512],
                                start=(k == 0),
                                stop=(k == KI - 1),
                            )
                        osb = osb_pool.tile([P, 512], f32, tag="osb")
                        nc.vector.tensor_copy(osb[:], po[:])
                        nc.sync.dma_start(
                            out_ap[dt, :, tb * 512:(tb + 1) * 512],
                            osb[:],
                        )

            # ---------------- Phase 1: g/u matmuls + silu*mul ----------------
            for h in range(NH):
                # Critical path first: mi=0 weights + first x tile, so the
                # first matmul chain can start as soon as possible; the
                # remaining x tiles stream behind it. The first gw tile is
                # split into k-quarters so chain 0 starts after ~1KB/part.
                gw0 = w_pool.tile([P, KD, P], bf16, tag="gw")
                uw0 = w_pool.tile([P, KD, P], bf16, tag="uw")
                xk = []
                if not skip_dma:
                    xt = xk_pool.tile([P, TH], bf16, tag="xk")
                    nc.sync.dma_start(xt[:], xt_ap[h, 0])
                    xk.append(xt)
                    nc.sync.dma_start(gw0[:], gw_ap[0])
                    nc.sync.dma_start(uw0[:], uw_ap[0])
                for k in range(len(xk), KD):
                    xt = xk_pool.tile([P, TH], bf16, tag="xk")
                    if not skip_dma:
                        nc.sync.dma_start(xt[:], xt_ap[h, k])
                    xk.append(xt)

                for mi in range(NMI):
                    if mi == 0:
                        gw, uw = gw0, uw0
                    else:
                        gw = w_pool.tile([P, KD, P], bf16, tag="gw")
                        uw = w_pool.tile([P, KD, P], bf16, tag="uw")
                        if not skip_dma:
                            nc.sync.dma_start(gw[:], gw_ap[mi])
                            nc.sync.dma_start(uw[:], uw_ap[mi])

                    pg = []
                    pu = []
                    for tb in range(NTB):
                        ps = ps_pool.tile([P, 512], f32, tag="ps")
                        if not skip_mm:
                            for k in range(KD):
                                nc.tensor.matmul(
                                    ps[:],
                                    gw[:, k, :],
                                    xk[k][:, tb * 512:(tb + 1) * 512],
                                    start=(k == 0),
                                    stop=(k == KD - 1),
                                )
                        pg.append(ps)
                    for tb in range(NTB):
                        ps = ps_pool.tile([P, 512], f32, tag="ps")
                        if not skip_mm:
                            for k in range(KD):
                                nc.tensor.matmul(
                                    ps[:],
                                    uw[:, k, :],
                                    xk[k][:, tb * 512:(tb + 1) * 512],
                                    start=(k == 0),
                                    stop=(k == KD - 1),
                                )
                        pu.append(ps)
                    if not skip_mm:
                        for tb in range(NTB):
                            tmp = tmp_pool.tile([P, 512], f32, tag="tmp")
                            nc.scalar.activation(
                                tmp[:], pg[tb][:],
                                mybir.ActivationFunctionType.Silu,
                            )
                            t0 = h * TH + tb * 512
                            nc.vector.tensor_tensor(
                                ht[:, mi, t0:t0 + 512],
                                tmp[:],
                                pu[tb][:],
                                mybir.AluOpType.mult,
                            )

                if h == 0:
                    # Interleave dt=0's first-half out-chains here: they only
                    # need half-0's ht, and give the PE ~14us of work while
                    # half 1's x tiles reclaim SBUF slots from half 0 (their
                    # DMAs can't start earlier, so without this the PE would
                    # briefly stall at the half boundary).
                    phase2_chains(0, (0, 1))

            # ---------------- Phase 2: out^T = ow^T-stationary @ h^T ---------
            phase2_chains(0, (2, 3))
            for dt in range(1, NDT):
                phase2_chains(dt, range(T // 512))

    nc.compile()
    return nc


def _get_program():
    global _COMPILED
    if _COMPILED is None:
        _COMPILED = _build_program()
    return _COMPILED


def _pack_inputs(x, gate_proj, inner_proj, output_proj):
    bf16 = ml_dtypes.bfloat16
    in_maps = []
    for e in range(E):
        # x [T, D] -> xT tiles [NH, KD, P, TH]: xt[h,k,p,tt] = x[h*TH+tt, k*P+p]
        xt = np.ascontiguousarray(x[e].T).astype(bf16)      # [D, T]
        xt = xt.reshape(KD, P, NH, TH).transpose(2, 0, 1, 3)
        xt = np.ascontiguousarray(xt)
        # gate [D, I] -> [NMI, P, KD, P]: gw[mi,p,k,ii] = gate[k*P+p, mi*P+ii]
        gw = gate_proj[e].astype(bf16).reshape(KD, P, NMI, P).transpose(2, 1, 0, 3)
        gw = np.ascontiguousarray(gw)
        uw = inner_proj[e].astype(bf16).reshape(KD, P, NMI, P).transpose(2, 1, 0, 3)
        uw = np.ascontiguousarray(uw)
        # outp [I, D] -> [NDT, P, KI, P]: ow[dt,p,k,dd] = outp[k*P+p, dt*P+dd]
        ow = output_proj[e].astype(bf16).reshape(KI, P, NDT, P).transpose(2, 1, 0, 3)
        ow = np.ascontiguousarray(ow)
        in_maps.append({"xt": xt, "gw": gw, "uw": uw, "ow": ow})
    return in_maps


def kernel(x, gate_proj, inner_proj, output_proj, _trace=False, _trace_kwargs=None):
    from concourse import bass_utils

    nc = _get_program()
    in_maps = _pack_inputs(
        np.asarray(x), np.asarray(gate_proj), np.asarray(inner_proj),
        np.asarray(output_proj),
    )
    res = bass_utils.run_bass_kernel_spmd(
        nc,
        in_maps,
        core_ids=list(range(E)),
        trace=_trace,
        **(_trace_kwargs or {}),
    )
    # device output is out^T [D, T]; transpose back per expert
    out = np.stack(
        [np.ascontiguousarray(np.asarray(res.results[e]["out"]).T) for e in range(E)]
    )
    if _trace:
        return out.astype(np.float32, copy=False), res
    return out.astype(np.float32, copy=False)


# revision 9
# speedup vs baseline: 1.0020x; 1.0009x over previous
"""GroupedExpertNetwork (SwiGLU per-expert MLP) Trainium2 kernel.

Expert-parallel: 8 experts -> 8 NeuronCores, one expert per core.
Per core:  g = x @ gate; u = x @ inner; h = silu(g)*u; out = h @ outp
Shapes per expert: x [T=2048, D=2048], gate/inner [D, I=4096], outp [I, D].

Low-HBM-traffic structure (109 MB/core vs 227 MB for the naive T-blocked
version), while keeping the PE stream dense at the bf16 roofline:

- Phase 1 (per T-half of 1024): for each i-tile (128 cols of gate/inner),
  run weight-stationary accumulation chains over both 512-token blocks of
  the half. Weights are streamed twice total (once per half) instead of
  once per 512-token block. xT stays resident as 16 per-k tiles per half.
  h^T [I, T] is materialized fully in SBUF (128 KB/partition-slice, bf16).
- Phase 2 (full T): output-projection-stationary: lhsT = ow [i, d-tile]
  chunks, moving = h^T columns. ow is streamed exactly once. This yields
  out^T [D, T] on the device; the host transposes back (cheap).

All matmul free dims are 512 (one PSUM bank), bf16 inputs, fp32 PSUM
accumulate. One shared 8-bank PSUM pool cycles through phase-1 g/u chains
and phase-2 out chains.
"""

import numpy as np
import ml_dtypes

E, T, D, I = 8, 2048, 2048, 4096
P = 128
TH = 1024                # T half for phase 1
NH = T // TH             # 2 halves
NTB = TH // 512          # 2 512-blocks per half
KD = D // P              # 16 contraction chunks (first layer)
KI = I // P              # 32 contraction chunks (second layer)
NMI = I // P             # 32 i-tiles (phase 1 output tiles)
NDT = D // P             # 16 d-tiles (phase 2 output tiles)

_COMPILED = None


def _build_program(skip_mm=False, skip_dma=False):
    import concourse.mybir as mybir
    import concourse.tile as tile
    from concourse import bacc

    bf16 = mybir.dt.bfloat16
    f32 = mybir.dt.float32

    nc = bacc.Bacc(
        "TRN2",
        target_bir_lowering=False,
        debug=False,
        num_devices=E,
    )

    # Packed DRAM inputs (per core = one expert):
    # xt:  [NH, KD, P, TH]    x^T tiles; d = k*128+p, t = h*1024 + tt
    # gw:  [NMI, P, KD, P]    gate tiles;  [mi, p(d), k, i-col]
    # uw:  [NMI, P, KD, P]    inner tiles
    # ow:  [NDT, P, KI, P]    output-proj tiles; [dt, p(i), k, d-col]
    xt_d = nc.dram_tensor("xt", (NH, KD, P, TH), bf16, kind="ExternalInput")
    gw_d = nc.dram_tensor("gw", (NMI, P, KD, P), bf16, kind="ExternalInput")
    uw_d = nc.dram_tensor("uw", (NMI, P, KD, P), bf16, kind="ExternalInput")
    ow_d = nc.dram_tensor("ow", (NDT, P, KI, P), bf16, kind="ExternalInput")
    # out^T [D, T] in bf16 (host upcasts + transposes back to [T, D];
    # bf16 rounding adds ~0.05% RMS against a 2% budget, halves out traffic)
    out_d = nc.dram_tensor("out", (D, T), bf16, kind="ExternalOutput")

    xt_ap = xt_d.ap()
    gw_ap = gw_d.ap()
    uw_ap = uw_d.ap()
    ow_ap = ow_d.ap()
    out_ap = out_d.ap().rearrange("(dt p) t -> dt p t", p=P)

    with tile.TileContext(nc) as tc:
        with (
            tc.tile_pool(name="xk", bufs=18) as xk_pool,
            tc.tile_pool(name="w", bufs=2) as w_pool,
            tc.tile_pool(name="ow", bufs=2) as ow_pool,
            tc.tile_pool(name="ht", bufs=1) as ht_pool,
            tc.tile_pool(name="tmp", bufs=2) as tmp_pool,
            tc.tile_pool(name="osb", bufs=2) as osb_pool,
            tc.tile_pool(name="ps", bufs=8, space="PSUM") as ps_pool,
        ):
            # h^T, fully resident: [p(i), ki, t]
            ht = ht_pool.tile([P, KI, T], bf16, tag="ht")

            def phase2_chains(dt, tbs):
                ow = ow_pool.tile([P, KI, P], bf16, tag="ow")
                if not skip_dma:
                    nc.sync.dma_start(ow[:], ow_ap[dt])
                for tb in tbs:
                    po = ps_pool.tile([P, 512], f32, tag="ps")
                    if not skip_mm:
                        for k in range(KI):
                            nc.tensor.matmul(
                                po[:],
                                ow[:, k, :],
                                ht[:, k, tb * 512:(tb + 1) * 512],
                                start=(k == 0),
                                stop=(k == KI - 1),
                            )
                        osb = osb_pool.tile([P, 512], bf16, tag="osb")
                        nc.vector.tensor_copy(osb[:], po[:])
                        nc.sync.dma_start(
                            out_ap[dt, :, tb * 512:(tb + 1) * 512],
                            osb[:],
                        )

            # ---------------- Phase 1: g/u matmuls + silu*mul ----------------
            for h in range(NH):
                # Critical path first: mi=0 weights + first x tile, so the
                # first matmul chain can start as soon as possible; the
                # remaining x tiles stream behind it. The first gw tile is
                # split into k-quarters so chain 0 starts after ~1KB/part.
                gw0 = w_pool.tile([P, KD, P], bf16, tag="gw")
                uw0 = w_pool.tile([P, KD, P], bf16, tag="uw")
                xk = []
                if not skip_dma:
                    xt = xk_pool.tile([P, TH], bf16, tag="xk")
                    nc.sync.dma_start(xt[:], xt_ap[h, 0])
                    xk.append(xt)
                    nc.sync.dma_start(gw0[:], gw_ap[0])
                    nc.sync.dma_start(uw0[:], uw_ap[0])
                for k in range(len(xk), KD):
                    xt = xk_pool.tile([P, TH], bf16, tag="xk")
                    if not skip_dma:
                        nc.sync.dma_start(xt[:], xt_ap[h, k])
                    xk.append(xt)

                for mi in range(NMI):
                    if mi == 0:
                        gw, uw = gw0, uw0
                    else:
                        gw = w_pool.tile([P, KD, P], bf16, tag="gw")
                        uw = w_pool.tile([P, KD, P], bf16, tag="uw")
                        if not skip_dma:
                            nc.sync.dma_start(gw[:], gw_ap[mi])
                            nc.sync.dma_start(uw[:], uw_ap[mi])

                    pg = []
                    pu = []
                    for tb in range(NTB):
                        ps = ps_pool.tile([P, 512], f32, tag="ps")
                        if not skip_mm:
                            for k in range(KD):
                                nc.tensor.matmul(
                                    ps[:],
                                    gw[:, k, :],
                                    xk[k][:, tb * 512:(tb + 1) * 512],
                                    start=(k == 0),
                                    stop=(k == KD - 1),
                                )
                        pg.append(ps)
                    for tb in range(NTB):
                        ps = ps_pool.tile([P, 512], f32, tag="ps")
                        if not skip_mm:
                            for k in range(KD):
                                nc.tensor.matmul(
                                    ps[:],
                                    uw[:, k, :],
                                    xk[k][:, tb * 512:(tb + 1) * 512],
                                    start=(k == 0),
                                    stop=(k == KD - 1),
                                )
                        pu.append(ps)
                    if not skip_mm:
                        for tb in range(NTB):
                            tmp = tmp_pool.tile([P, 512], f32, tag="tmp")
                            nc.scalar.activation(
                                tmp[:], pg[tb][:],
                                mybir.ActivationFunctionType.Silu,
                            )
                            t0 = h * TH + tb * 512
                            nc.vector.tensor_tensor(
                                ht[:, mi, t0:t0 + 512],
                                tmp[:],
                                pu[tb][:],
                                mybir.AluOpType.mult,
                            )

                if h == 0:
                    # Interleave dt=0's first-half out-chains here: they only
                    # need half-0's ht, and give the PE ~14us of work while
                    # half 1's x tiles reclaim SBUF slots from half 0 (their
                    # DMAs can't start earlier, so without this the PE would
                    # briefly stall at the half boundary).
                    phase2_chains(0, (0, 1))

            # ---------------- Phase 2: out^T = ow^T-stationary @ h^T ---------
            phase2_chains(0, (2, 3))
            for dt in range(1, NDT):
                phase2_chains(dt, range(T // 512))

    nc.compile()
    return nc


def _get_program():
    global _COMPILED
    if _COMPILED is None:
        _COMPILED = _build_program()
    return _COMPILED


def _pack_inputs(x, gate_proj, inner_proj, output_proj):
    bf16 = ml_dtypes.bfloat16
    in_maps = []
    for e in range(E):
        # x [T, D] -> xT tiles [NH, KD, P, TH]: xt[h,k,p,tt] = x[h*TH+tt, k*P+p]
        xt = np.ascontiguousarray(x[e].T).astype(bf16)      # [D, T]
        xt = xt.reshape(KD, P, NH, TH).transpose(2, 0, 1, 3)
        xt = np.ascontiguousarray(xt)
        # gate [D, I] -> [NMI, P, KD, P]: gw[mi,p,k,ii] = gate[k*P+p, mi*P+ii]
        gw = gate_proj[e].astype(bf16).reshape(KD, P, NMI, P).transpose(2, 1, 0, 3)
        gw = np.ascontiguousarray(gw)
        uw = inner_proj[e].astype(bf16).reshape(KD, P, NMI, P).transpose(2, 1, 0, 3)
        uw = np.ascontiguousarray(uw)
        # outp [I, D] -> [NDT, P, KI, P]: ow[dt,p,k,dd] = outp[k*P+p, dt*P+dd]
        ow = output_proj[e].astype(bf16).reshape(KI, P, NDT, P).transpose(2, 1, 0, 3)
        ow = np.ascontiguousarray(ow)
        in_maps.append({"xt": xt, "gw": gw, "uw": uw, "ow": ow})
    return in_maps


def kernel(x, gate_proj, inner_proj, output_proj, _trace=False, _trace_kwargs=None):
    from concourse import bass_utils

    nc = _get_program()
    in_maps = _pack_inputs(
        np.asarray(x), np.asarray(gate_proj), np.asarray(inner_proj),
        np.asarray(output_proj),
    )
    res = bass_utils.run_bass_kernel_spmd(
        nc,
        in_maps,
        core_ids=list(range(E)),
        trace=_trace,
        **(_trace_kwargs or {}),
    )
    # device output is out^T [D, T]; transpose back per expert
    out = np.stack(
        [np.ascontiguousarray(np.asarray(res.results[e]["out"]).T) for e in range(E)]
    )
    if _trace:
        return out.astype(np.float32, copy=False), res
    return out.astype(np.float32, copy=False)
